# revision 13
# baseline (speedup 1.0000x reference)
"""Trainium2 Bass kernel for nn_CLF_block (channel-attention block).

Reference computation (per batch item, with x = concat([a,b], ch) in [256, N],
N = H*W = 16384):
    z  = w1 x + b1 1^T
    q  = w2 z + b2 1^T ;  k = w3 z + b3 1^T ;  v = w4 z + b4 1^T
    qk = q k^T ; attn = softmax(qk, -1) ; out = attn v

Weight folding (host): A = w2 w1, B = w3 w1, Cw = w4 w1,
beta2 = w2 b1 + b2, beta3 = w3 b1 + b3, beta4 = w4 b1 + b4.  Then with
Gx = x x^T and sx = x 1 (one streaming pass over x):
    qk   = A Gx B^T + (A sx) beta3^T + beta2 (B sx)^T + N beta2 beta3^T
    attn = softmax(qk)
    W    = attn Cw ; c0 = attn beta4
    out  = W x + c0 1^T          (second streaming pass over x)

Data plan (per core; tolerance is 2e-2 so fp16 x suffices end-to-end,
numpy-verified rel err ~2.9e-3):
  - x streams once as transposed fp16 pieces xht [n, c] with a ones column
    (8.4 MiB, 8 KiB DMA lines); the Gram accumulates directly from them.
  - x also streams once in natural fp16 layout (8.4 MiB, 8 KiB lines) as the
    pass-2 moving operand (cheaper than on-chip PE transposes, which starve
    the PE sequencer).
  - Output is written as fp16 (8.4 MiB, 4 KiB lines) and upcast on host.

Sharding: data-parallel over batch, one batch item per NeuronCore (B=8).
"""

import sys

if "/opt/trn_rl_repo" not in sys.path:
    sys.path.insert(0, "/opt/trn_rl_repo")

from contextlib import ExitStack

import numpy as np

import concourse.bass as bass
import concourse.mybir as mybir
import concourse.tile as tile
from concourse import bacc
from concourse.bass_utils import run_bass_kernel_spmd

F32 = mybir.dt.float32
F32R = mybir.dt.float32r
F16 = mybir.dt.float16
P = 128          # partitions / channel block
C = 256          # channels
NPIX = 128 * 128  # spatial positions per batch item
NPIECE = 8        # streamed x^T pieces
PIECE = NPIX // NPIECE   # 2048 cols per piece
NCHUNK = NPIX // P       # 128 gram chunks
CH_PP = PIECE // P       # gram chunks per piece (16)
NT = 512          # matmul moving-operand width for pass 2
OG = 2048         # output staging width (4 KiB fp16 lines)


def _emit(nc, tc, ctx, d_in, d_out):
    """Emit the Tile program for one core (one batch item)."""
    wcat, ident = d_in["wcat"], d_in["ident"]
    xht_d, xnat_d = d_in["xht"], d_in["xnat"]
    brows, bcols = d_in["brows"], d_in["bcols"]
    out_d = d_out["out"]

    const = ctx.enter_context(tc.tile_pool(name="const", bufs=1))
    xpool = ctx.enter_context(tc.tile_pool(name="xpool", bufs=1))

    qs = [nc.sync, nc.scalar]

    # --- constants -------------------------------------------------------
    # wcat columns: [A^T | B^T | Cw] as two 128-row blocks, f32r for the
    # fast-path PE matmuls (1 cyc/row at >=256 moving columns).
    w_sb = []
    for k in range(2):
        wt = const.tile([P, 3 * C], F32, name=f"w_sb{k}", tag=f"w_sb{k}")
        nc.sync.dma_start(out=wt, in_=wcat[k * P:(k + 1) * P, :])
        w_sb.append(wt)
    at_ = [w_sb[k][:, 0 * C:1 * C] for k in range(2)]   # A^T  [c', o]
    bt_ = [w_sb[k][:, 1 * C:2 * C] for k in range(2)]   # B^T  [c', o]
    cw_ = [w_sb[k][:, 2 * C:3 * C] for k in range(2)]   # Cw   [d, c']

    rows = []
    for r in range(3):
        rt = const.tile([1, C], F32, name=f"brow{r}", tag=f"brow{r}")
        nc.sync.dma_start(out=rt, in_=brows[r:r + 1, :])
        rows.append(rt)
    b2_row, b3_row, nb3_row = rows

    b4_col = []
    for k in range(2):
        bt = const.tile([P, 1], F32, name=f"bcol{k}", tag=f"bcol{k}")
        nc.sync.dma_start(out=bt, in_=bcols[k * P:(k + 1) * P, :])
        b4_col.append(bt)

    ident_sb = const.tile([P, P], F32, name="ident_sb", tag="ident_sb")
    nc.sync.dma_start(out=ident_sb, in_=ident[:, :])

    # --- natural-layout fp16 x (pass-2 moving operand), 4 big tiles/block
    spj = NPIX // 4         # columns per resident x tile
    xs = [[], []]
    for k in range(2):
        for j in range(4):
            xt = xpool.tile([P, spj], F16, name=f"x{k}_{j}", tag=f"x{k}_{j}")
            qs[(k * 4 + j) % 2].dma_start(
                out=xt, in_=xnat_d[k, :, j * spj:(j + 1) * spj])
            xs[k].append(xt)

    # --- pass 1: stream x^T pieces; Gram [both 128-row blocks, full width]
    gx_sb = [
        const.tile([P, C + 1], F32, name=f"gx_sb{b}", tag=f"gx_sb{b}")
        for b in range(2)
    ]
    with tc.tile_pool(name="gx_ps", bufs=1, space="PSUM") as gxp, \
         tc.tile_pool(name="xt_sb", bufs=3) as xtp:
        shh = [
            gxp.tile([P, C + 1], F32, name=f"shh{b}", tag=f"shh{b}")
            for b in range(2)
        ]
        for i in range(NPIECE):
            xht_p = xtp.tile([P, CH_PP, C + 1], F16, name="xht_p", tag="xht_p")
            qs[i % 2].dma_start(out=xht_p, in_=xht_d[i])
            for g in range(CH_PP):
                ch = i * CH_PP + g
                chunk = xht_p[:, g, :]
                for b in range(2):
                    nc.tensor.matmul(shh[b], chunk[:, b * P:(b + 1) * P],
                                     chunk,
                                     start=(ch == 0), stop=(ch == NCHUNK - 1))
        nc.vector.tensor_copy(gx_sb[0], shh[0])
        nc.vector.tensor_copy(gx_sb[1], shh[1])

    # Split the (large) diagonal out of Gx so the f32r algebra matmuls see
    # well-scaled operands; the diagonal term is re-applied exactly via
    # per-partition multiplies.
    gxd = []
    for b in range(2):
        bs = slice(b * P, (b + 1) * P)
        dm = const.tile([P, P], F32, name=f"gxdm{b}", tag=f"gxdm{b}")
        nc.vector.tensor_mul(dm, gx_sb[b][:, bs], ident_sb)
        dcol = const.tile([P, 1], F32, name=f"gxd{b}", tag=f"gxd{b}")
        nc.vector.reduce_sum(out=dcol, in_=dm, axis=mybir.AxisListType.X)
        nc.vector.tensor_sub(gx_sb[b][:, bs],
                             gx_sb[b][:, bs], dm)
        gxd.append(dcol)

    # --- tiny 256x256 algebra -------------------------------------------
    alg_sb = const

    with tc.tile_pool(name="alg_ps", bufs=3, space="PSUM") as ap:
        # asx = (A sx)^T, bsx = (B sx)^T (sx sits in gx col 256)
        asx_row = alg_sb.tile([1, C], F32, name="asx_row", tag="asx_row")
        bsx_row = alg_sb.tile([1, C], F32, name="bsx_row", tag="bsx_row")
        for dst, wt in ((asx_row, at_), (bsx_row, bt_)):
            vps = ap.tile([1, C], F32, name="vps", tag="algsmall", bufs=2)
            for k in range(2):
                nc.tensor.matmul(vps, gx_sb[k][:, C:C + 1], wt[k],
                                 start=(k == 0), stop=(k == 1))
            nc.vector.tensor_copy(dst, vps)

        # U2 = (A Gx)^T = Gx~ A^T + D A^T
        u2_sb = []
        for b in range(2):
            u2ps = ap.tile([P, C], F32, name="u2ps", tag="alg")
            for k in range(2):
                nc.tensor.matmul(u2ps, gx_sb[k][:, b * P:(b + 1) * P], at_[k],
                                 start=(k == 0), stop=(k == 1))
            u2d = alg_sb.tile([P, C], F32, name=f"u2_d{b}", tag=f"u2_d{b}")
            nc.vector.tensor_scalar_mul(u2d, at_[b], gxd[b])
            u2t = alg_sb.tile([P, C], F32, name=f"u2_sb{b}", tag=f"u2_sb{b}")
            nc.vector.tensor_add(u2t, u2ps, u2d)
            u2_sb.append(u2t)

        # qk = U2^T B^T + rank-1 terms ; then softmax rows
        attn_sb = []
        for b in range(2):
            qkps = ap.tile([P, C], F32, name="qkps", tag="alg")
            for k in range(2):
                nc.tensor.matmul(qkps, u2_sb[k][:, b * P:(b + 1) * P], bt_[k],
                                 start=(k == 0), stop=False)
            nc.tensor.matmul(qkps, asx_row[:, b * P:(b + 1) * P], b3_row,
                             start=False, stop=False)
            nc.tensor.matmul(qkps, b2_row[:, b * P:(b + 1) * P], bsx_row,
                             start=False, stop=False)
            nc.tensor.matmul(qkps, b2_row[:, b * P:(b + 1) * P],
                             nb3_row, start=False, stop=True)

            negmax = alg_sb.tile([P, 1], F32, name=f"negmax{b}", tag=f"nm{b}")
            nc.vector.tensor_reduce(
                out=negmax, in_=qkps, op=mybir.AluOpType.max,
                axis=mybir.AxisListType.X, negate=True,
            )
            expq = alg_sb.tile([P, C], F32, name=f"expq{b}", tag=f"expq{b}")
            nc.scalar.activation(
                out=expq, in_=qkps, func=mybir.ActivationFunctionType.Exp,
                bias=negmax, scale=1.0,
            )
            denom = alg_sb.tile([P, 1], F32, name=f"denom{b}", tag=f"dn{b}")
            nc.vector.reduce_sum(out=denom, in_=expq,
                                 axis=mybir.AxisListType.X)
            rden = alg_sb.tile([P, 1], F32, name=f"rden{b}", tag=f"rd{b}")
            nc.vector.reciprocal(rden, denom)
            at = alg_sb.tile([P, C], F32, name=f"attn{b}", tag=f"attn{b}")
            nc.vector.tensor_scalar_mul(at, expq, rden)
            attn_sb.append(at)

        # attn^T (4 PE transposes)
        attnT_sb = [
            alg_sb.tile([P, C], F32, name=f"attnT{j}", tag=f"attnT{j}")
            for j in range(2)
        ]
        for b in range(2):
            for j in range(2):
                tps = ap.tile([P, P], F32, name="tps", tag="algtp", bufs=2)
                nc.tensor.transpose(tps, attn_sb[b][:, j * P:(j + 1) * P],
                                    ident_sb)
                nc.vector.tensor_copy(
                    attnT_sb[j][:, b * P:(b + 1) * P], tps)

        # W^T = Cw-as-lhsT @ attn^T  (stored fp16 for the pass-2 matmuls)
        wt_sb = []
        for b in range(2):
            wps = ap.tile([P, C], F32, name="wps", tag="alg")
            for k in range(2):
                nc.tensor.matmul(wps, cw_[k][:, b * P:(b + 1) * P],
                                 attnT_sb[k], start=(k == 0), stop=(k == 1))
            wt_ = alg_sb.tile([P, C], F16, name=f"wt_sb{b}", tag=f"wt_sb{b}")
            nc.vector.tensor_copy(wt_, wps)
            wt_sb.append(wt_)

        # c0_col = attn beta4 (per block)
        c0_col = []
        for b in range(2):
            cps = ap.tile([P, 1], F32, name="cps", tag="algsmall", bufs=2)
            for k in range(2):
                nc.tensor.matmul(cps,
                                 attnT_sb[k][:, b * P:(b + 1) * P],
                                 b4_col[k], start=(k == 0), stop=(k == 1))
            ct = alg_sb.tile([P, 1], F32, name=f"c0_col{b}", tag=f"c0_col{b}")
            nc.scalar.copy(ct, cps)
            c0_col.append(ct)

    # --- pass 2: out = W x + c0 1^T, fp16 out, 4 KiB DMA lines -----------
    with tc.tile_pool(name="o_ps", bufs=8, space="PSUM") as ops, \
         tc.tile_pool(name="o_sb", bufs=3) as osb:
        ngrp = NPIX // OG       # 8 output groups of OG columns
        nsub = OG // NT         # 4 psum tiles per staging tile
        for i in range(ngrp):
            for b in range(2):
                ot = osb.tile([P, OG], F16, name="ot", tag="ot")
                pst = [
                    ops.tile([P, NT], F32, name="pst", tag="pst")
                    for _ in range(nsub)
                ]
                for k in range(2):
                    for t in range(nsub):
                        col = i * OG + t * NT
                        nc.tensor.matmul(
                            pst[t],
                            wt_sb[k][:, b * P:(b + 1) * P],
                            xs[k][col // spj][:, col % spj:col % spj + NT],
                            start=(k == 0),
                            stop=(k == 1),
                        )
                for t in range(nsub):
                    eng = (nc.scalar, nc.vector)[(i + b + t) % 2]
                    if eng is nc.scalar:
                        eng.activation(
                            out=ot[:, t * NT:(t + 1) * NT], in_=pst[t],
                            func=mybir.ActivationFunctionType.Identity,
                            bias=c0_col[b], scale=1.0,
                        )
                    else:
                        eng.tensor_scalar_add(ot[:, t * NT:(t + 1) * NT],
                                              pst[t], c0_col[b])
                qs[(2 * i + b) % 2].dma_start(
                    out=out_d[b * P:(b + 1) * P, i * OG:(i + 1) * OG],
                    in_=ot,
                )


def build_program(enable_asserts=False):
    nc = bacc.Bacc(
        "TRN2",
        target_bir_lowering=False,
        debug=False,
        enable_asserts=enable_asserts,
        num_devices=8,
    )
    d_in = {
        "xht": nc.dram_tensor("xht", [NPIECE, P, CH_PP, C + 1],
                              F16, kind="ExternalInput").ap(),
        "xnat": nc.dram_tensor("xnat", [2, P, NPIX], F16,
                               kind="ExternalInput").ap(),
        "wcat": nc.dram_tensor("wcat", [C, 3 * C], F32,
                               kind="ExternalInput").ap(),
        "brows": nc.dram_tensor("brows", [3, C], F32,
                                kind="ExternalInput").ap(),
        "bcols": nc.dram_tensor("bcols", [C, 1], F32,
                                kind="ExternalInput").ap(),
        "ident": nc.dram_tensor("ident", [P, P], F32,
                                kind="ExternalInput").ap(),
    }
    d_out = {
        "out": nc.dram_tensor("out", [C, NPIX], F16,
                              kind="ExternalOutput").ap(),
    }
    with tile.TileContext(nc) as tc, ExitStack() as ctx:
        _emit(nc, tc, ctx, d_in, d_out)
    nc.compile()
    return nc


def _round_f32r(x):
    """Round fp32 to the FP32R-representable set (hi-bf16 + lo-bf16)."""
    import ml_dtypes

    x = np.asarray(x, np.float32)
    hi = x.astype(ml_dtypes.bfloat16).astype(np.float32)
    lo = (x - hi).astype(ml_dtypes.bfloat16).astype(np.float32)
    return hi + lo


def make_in_maps(a, b, w1, b1, w2, b2, w3, b3, w4, b4):
    N = NPIX
    f = np.float32
    A = (w2.astype(np.float64) @ w1.astype(np.float64)).astype(f)
    B_ = (w3.astype(np.float64) @ w1.astype(np.float64)).astype(f)
    Cw = (w4.astype(np.float64) @ w1.astype(np.float64)).astype(f)
    be2 = (w2 @ b1 + b2).astype(f)
    be3 = (w3 @ b1 + b3).astype(f)
    be4 = (w4 @ b1 + b4).astype(f)
    wcat = np.concatenate([A.T, B_.T, Cw], axis=1).astype(f, copy=False)
    brows = np.stack([be2, be3, N * be3]).astype(f, copy=False)
    bcols = np.ascontiguousarray(be4[:, None].astype(f))
    ident = np.eye(P, dtype=f)
    in_maps = []
    for i in range(a.shape[0]):
        x = np.concatenate([a[i].reshape(P, N), b[i].reshape(P, N)], axis=0)
        xh = x.astype(np.float16)
        xht = np.ascontiguousarray(
            xh.T.reshape(NPIECE, CH_PP, P, C).transpose(0, 2, 1, 3))
        ones = np.ones((NPIECE, P, CH_PP, 1), np.float16)
        xht = np.ascontiguousarray(np.concatenate([xht, ones], axis=3))
        in_maps.append({
            "xht": xht,
            "xnat": np.ascontiguousarray(xh.reshape(2, P, N)),
            "wcat": wcat,
            "brows": brows,
            "bcols": bcols,
            "ident": ident,
        })
    return in_maps


_CACHE = {}


def kernel(a, b, w1, b1, w2, b2, w3, b3, w4, b4, _trace=False):
    a = np.asarray(a, dtype=np.float32)
    b = np.asarray(b, dtype=np.float32)
    args = [np.asarray(t, dtype=np.float32)
            for t in (w1, b1, w2, b2, w3, b3, w4, b4)]
    if "nc" not in _CACHE:
        _CACHE["nc"] = build_program()
    nc = _CACHE["nc"]
    in_maps = make_in_maps(a, b, *args)
    res = run_bass_kernel_spmd(nc, in_maps, core_ids=list(range(8)),
                               trace=_trace)
    B, Ch, H, W = a.shape
    out = np.stack([np.asarray(r["out"], dtype=np.float32).reshape(C, H, W)
                    for r in res.results])
    if _trace:
        _CACHE["last_results"] = res
    return out


# revision 14
# speedup vs baseline: 1.1211x; 1.1211x over previous
"""Trainium2 Bass kernel for nn_CLF_block (channel-attention block).

Reference computation (per batch item, with x = concat([a,b], ch) in [256, N],
N = H*W = 16384):
    z  = w1 x + b1 1^T
    q  = w2 z + b2 1^T ;  k = w3 z + b3 1^T ;  v = w4 z + b4 1^T
    qk = q k^T ; attn = softmax(qk, -1) ; out = attn v

Weight folding (host): A = w2 w1, B = w3 w1, Cw = w4 w1,
beta2 = w2 b1 + b2, beta3 = w3 b1 + b3, beta4 = w4 b1 + b4.  Then with
Gx = x x^T and sx = x 1 (one streaming pass over x):
    qk   = A Gx B^T + (A sx) beta3^T + beta2 (B sx)^T + N beta2 beta3^T
    attn = softmax(qk)
    W    = attn Cw ; c0 = attn beta4
    out  = W x + c0 1^T          (second streaming pass over x)

Data plan (per core; tolerance is 2e-2 so fp16 x suffices end-to-end,
numpy-verified rel err ~2.9e-3):
  - x streams once as transposed fp16 pieces xht [n, c] with a ones column
    (8.4 MiB, 8 KiB DMA lines); the Gram accumulates directly from them.
  - x also streams once in natural fp16 layout (8.4 MiB, 8 KiB lines) as the
    pass-2 moving operand (cheaper than on-chip PE transposes, which starve
    the PE sequencer).
  - Output is written as fp16 (8.4 MiB, 4 KiB lines) and upcast on host.

Sharding: data-parallel over batch, one batch item per NeuronCore (B=8).
"""

import sys

if "/opt/trn_rl_repo" not in sys.path:
    sys.path.insert(0, "/opt/trn_rl_repo")

from contextlib import ExitStack

import numpy as np

import concourse.bass as bass
import concourse.mybir as mybir
import concourse.tile as tile
from concourse import bacc
from concourse.bass_utils import run_bass_kernel_spmd

F32 = mybir.dt.float32
F32R = mybir.dt.float32r
F16 = mybir.dt.float16
P = 128          # partitions / channel block
C = 256          # channels
NPIX = 128 * 128  # spatial positions per batch item
NPIECE = 8        # streamed x^T pieces
PIECE = NPIX // NPIECE   # 2048 cols per piece
NCHUNK = NPIX // P       # 128 gram chunks
CH_PP = PIECE // P       # gram chunks per piece (16)
NT = 512          # matmul moving-operand width for pass 2
OG = 2048         # output staging width (4 KiB fp16 lines)


def _emit(nc, tc, ctx, d_in, d_out):
    """Emit the Tile program for one core (one batch item)."""
    wcat, ident = d_in["wcat"], d_in["ident"]
    xht_d, xnat_d = d_in["xht"], d_in["xnat"]
    brows, bcols = d_in["brows"], d_in["bcols"]
    out_d = d_out["out"]

    const = ctx.enter_context(tc.tile_pool(name="const", bufs=1))
    xpool = ctx.enter_context(tc.tile_pool(name="xpool", bufs=1))

    qs = [nc.sync, nc.scalar]

    # --- constants -------------------------------------------------------
    # wcat columns: [A^T | B^T | Cw] as two 128-row blocks, f32r for the
    # fast-path PE matmuls (1 cyc/row at >=256 moving columns).
    w_sb = []
    for k in range(2):
        wt = const.tile([P, 3 * C], F32, name=f"w_sb{k}", tag=f"w_sb{k}")
        nc.sync.dma_start(out=wt, in_=wcat[k * P:(k + 1) * P, :])
        w_sb.append(wt)
    at_ = [w_sb[k][:, 0 * C:1 * C] for k in range(2)]   # A^T  [c', o]
    bt_ = [w_sb[k][:, 1 * C:2 * C] for k in range(2)]   # B^T  [c', o]
    cw_ = [w_sb[k][:, 2 * C:3 * C] for k in range(2)]   # Cw   [d, c']

    rows = []
    for r in range(3):
        rt = const.tile([1, C], F32, name=f"brow{r}", tag=f"brow{r}")
        nc.sync.dma_start(out=rt, in_=brows[r:r + 1, :])
        rows.append(rt)
    b2_row, b3_row, nb3_row = rows

    b4_col = []
    for k in range(2):
        bt = const.tile([P, 1], F32, name=f"bcol{k}", tag=f"bcol{k}")
        nc.sync.dma_start(out=bt, in_=bcols[k * P:(k + 1) * P, :])
        b4_col.append(bt)

    ident_sb = const.tile([P, P], F32, name="ident_sb", tag="ident_sb")
    nc.sync.dma_start(out=ident_sb, in_=ident[:, :])

    # --- pass 1: stream x^T pieces; Gram [both 128-row blocks, full width]
    gx_sb = [
        const.tile([P, C + 1], F32, name=f"gx_sb{b}", tag=f"gx_sb{b}")
        for b in range(2)
    ]
    with tc.tile_pool(name="gx_ps", bufs=1, space="PSUM") as gxp, \
         tc.tile_pool(name="xt_sb", bufs=3) as xtp:
        shh = [
            gxp.tile([P, C + 1], F32, name=f"shh{b}", tag=f"shh{b}")
            for b in range(2)
        ]
        for i in range(NPIECE):
            xht_p = xtp.tile([P, CH_PP, C + 1], F16, name="xht_p", tag="xht_p")
            qs[i % 2].dma_start(out=xht_p, in_=xht_d[i])
            for g in range(CH_PP):
                ch = i * CH_PP + g
                chunk = xht_p[:, g, :]
                for b in range(2):
                    nc.tensor.matmul(shh[b], chunk[:, b * P:(b + 1) * P],
                                     chunk,
                                     start=(ch == 0), stop=(ch == NCHUNK - 1))
        nc.vector.tensor_copy(gx_sb[0], shh[0])
        nc.vector.tensor_copy(gx_sb[1], shh[1])

    # natural-layout fp16 x (pass-2 moving operand): queued AFTER the x^T
    # stream so the Gram is never starved; finishes during the algebra.
    spj = NPIX // 4         # columns per resident x tile
    xs = [[], []]
    for k in range(2):
        for j in range(4):
            xt = xpool.tile([P, spj], F16, name=f"x{k}_{j}", tag=f"x{k}_{j}")
            qs[(k * 4 + j) % 2].dma_start(
                out=xt, in_=xnat_d[k, :, j * spj:(j + 1) * spj])
            xs[k].append(xt)

    # Split the (large) diagonal out of Gx so the f32r algebra matmuls see
    # well-scaled operands; the diagonal term is re-applied exactly via
    # per-partition multiplies.
    gxd = []
    for b in range(2):
        bs = slice(b * P, (b + 1) * P)
        dm = const.tile([P, P], F32, name=f"gxdm{b}", tag=f"gxdm{b}")
        nc.vector.tensor_mul(dm, gx_sb[b][:, bs], ident_sb)
        dcol = const.tile([P, 1], F32, name=f"gxd{b}", tag=f"gxd{b}")
        nc.vector.reduce_sum(out=dcol, in_=dm, axis=mybir.AxisListType.X)
        nc.vector.tensor_sub(gx_sb[b][:, bs],
                             gx_sb[b][:, bs], dm)
        gxd.append(dcol)

    # --- tiny 256x256 algebra -------------------------------------------
    alg_sb = const

    with tc.tile_pool(name="alg_ps", bufs=3, space="PSUM") as ap:
        # asx = (A sx)^T, bsx = (B sx)^T (sx sits in gx col 256)
        asx_row = alg_sb.tile([1, C], F32, name="asx_row", tag="asx_row")
        bsx_row = alg_sb.tile([1, C], F32, name="bsx_row", tag="bsx_row")
        for dst, wt in ((asx_row, at_), (bsx_row, bt_)):
            vps = ap.tile([1, C], F32, name="vps", tag="algsmall", bufs=2)
            for k in range(2):
                nc.tensor.matmul(vps, gx_sb[k][:, C:C + 1], wt[k],
                                 start=(k == 0), stop=(k == 1))
            nc.vector.tensor_copy(dst, vps)

        # U2 = (A Gx)^T = Gx~ A^T + D A^T
        u2_sb = []
        for b in range(2):
            u2ps = ap.tile([P, C], F32, name="u2ps", tag="alg")
            for k in range(2):
                nc.tensor.matmul(u2ps, gx_sb[k][:, b * P:(b + 1) * P], at_[k],
                                 start=(k == 0), stop=(k == 1))
            u2d = alg_sb.tile([P, C], F32, name=f"u2_d{b}", tag=f"u2_d{b}")
            nc.vector.tensor_scalar_mul(u2d, at_[b], gxd[b])
            u2t = alg_sb.tile([P, C], F32, name=f"u2_sb{b}", tag=f"u2_sb{b}")
            nc.vector.tensor_add(u2t, u2ps, u2d)
            u2_sb.append(u2t)

        # qk = U2^T B^T + rank-1 terms ; then softmax rows
        attn_sb = []
        for b in range(2):
            qkps = ap.tile([P, C], F32, name="qkps", tag="alg")
            for k in range(2):
                nc.tensor.matmul(qkps, u2_sb[k][:, b * P:(b + 1) * P], bt_[k],
                                 start=(k == 0), stop=False)
            nc.tensor.matmul(qkps, asx_row[:, b * P:(b + 1) * P], b3_row,
                             start=False, stop=False)
            nc.tensor.matmul(qkps, b2_row[:, b * P:(b + 1) * P], bsx_row,
                             start=False, stop=False)
            nc.tensor.matmul(qkps, b2_row[:, b * P:(b + 1) * P],
                             nb3_row, start=False, stop=True)

            negmax = alg_sb.tile([P, 1], F32, name=f"negmax{b}", tag=f"nm{b}")
            nc.vector.tensor_reduce(
                out=negmax, in_=qkps, op=mybir.AluOpType.max,
                axis=mybir.AxisListType.X, negate=True,
            )
            expq = alg_sb.tile([P, C], F32, name=f"expq{b}", tag=f"expq{b}")
            nc.scalar.activation(
                out=expq, in_=qkps, func=mybir.ActivationFunctionType.Exp,
                bias=negmax, scale=1.0,
            )
            denom = alg_sb.tile([P, 1], F32, name=f"denom{b}", tag=f"dn{b}")
            nc.vector.reduce_sum(out=denom, in_=expq,
                                 axis=mybir.AxisListType.X)
            rden = alg_sb.tile([P, 1], F32, name=f"rden{b}", tag=f"rd{b}")
            nc.vector.reciprocal(rden, denom)
            at = alg_sb.tile([P, C], F32, name=f"attn{b}", tag=f"attn{b}")
            nc.vector.tensor_scalar_mul(at, expq, rden)
            attn_sb.append(at)

        # attn^T (4 PE transposes)
        attnT_sb = [
            alg_sb.tile([P, C], F32, name=f"attnT{j}", tag=f"attnT{j}")
            for j in range(2)
        ]
        for b in range(2):
            for j in range(2):
                tps = ap.tile([P, P], F32, name="tps", tag="algtp", bufs=2)
                nc.tensor.transpose(tps, attn_sb[b][:, j * P:(j + 1) * P],
                                    ident_sb)
                nc.vector.tensor_copy(
                    attnT_sb[j][:, b * P:(b + 1) * P], tps)

        # W^T = Cw-as-lhsT @ attn^T  (stored fp16 for the pass-2 matmuls)
        wt_sb = []
        for b in range(2):
            wps = ap.tile([P, C], F32, name="wps", tag="alg")
            for k in range(2):
                nc.tensor.matmul(wps, cw_[k][:, b * P:(b + 1) * P],
                                 attnT_sb[k], start=(k == 0), stop=(k == 1))
            wt_ = alg_sb.tile([P, C], F16, name=f"wt_sb{b}", tag=f"wt_sb{b}")
            nc.vector.tensor_copy(wt_, wps)
            wt_sb.append(wt_)

        # c0_col = attn beta4 (per block)
        c0_col = []
        for b in range(2):
            cps = ap.tile([P, 1], F32, name="cps", tag="algsmall", bufs=2)
            for k in range(2):
                nc.tensor.matmul(cps,
                                 attnT_sb[k][:, b * P:(b + 1) * P],
                                 b4_col[k], start=(k == 0), stop=(k == 1))
            ct = alg_sb.tile([P, 1], F32, name=f"c0_col{b}", tag=f"c0_col{b}")
            nc.scalar.copy(ct, cps)
            c0_col.append(ct)

    # --- pass 2: out = W x + c0 1^T, fp16 out, 4 KiB DMA lines -----------
    with tc.tile_pool(name="o_ps", bufs=8, space="PSUM") as ops, \
         tc.tile_pool(name="o_sb", bufs=3) as osb:
        ngrp = NPIX // OG       # 8 output groups of OG columns
        nsub = OG // NT         # 4 psum tiles per staging tile
        for i in range(ngrp):
            for b in range(2):
                ot = osb.tile([P, OG], F16, name="ot", tag="ot")
                pst = [
                    ops.tile([P, NT], F32, name="pst", tag="pst")
                    for _ in range(nsub)
                ]
                for k in range(2):
                    for t in range(nsub):
                        col = i * OG + t * NT
                        nc.tensor.matmul(
                            pst[t],
                            wt_sb[k][:, b * P:(b + 1) * P],
                            xs[k][col // spj][:, col % spj:col % spj + NT],
                            start=(k == 0),
                            stop=(k == 1),
                        )
                for t in range(nsub):
                    eng = (nc.scalar, nc.vector)[(i + b + t) % 2]
                    if eng is nc.scalar:
                        eng.activation(
                            out=ot[:, t * NT:(t + 1) * NT], in_=pst[t],
                            func=mybir.ActivationFunctionType.Identity,
                            bias=c0_col[b], scale=1.0,
                        )
                    else:
                        eng.tensor_scalar_add(ot[:, t * NT:(t + 1) * NT],
                                              pst[t], c0_col[b])
                qs[(2 * i + b) % 2].dma_start(
                    out=out_d[b * P:(b + 1) * P, i * OG:(i + 1) * OG],
                    in_=ot,
                )


def build_program(enable_asserts=False):
    nc = bacc.Bacc(
        "TRN2",
        target_bir_lowering=False,
        debug=False,
        enable_asserts=enable_asserts,
        num_devices=8,
    )
    d_in = {
        "xht": nc.dram_tensor("xht", [NPIECE, P, CH_PP, C + 1],
                              F16, kind="ExternalInput").ap(),
        "xnat": nc.dram_tensor("xnat", [2, P, NPIX], F16,
                               kind="ExternalInput").ap(),
        "wcat": nc.dram_tensor("wcat", [C, 3 * C], F32,
                               kind="ExternalInput").ap(),
        "brows": nc.dram_tensor("brows", [3, C], F32,
                                kind="ExternalInput").ap(),
        "bcols": nc.dram_tensor("bcols", [C, 1], F32,
                                kind="ExternalInput").ap(),
        "ident": nc.dram_tensor("ident", [P, P], F32,
                                kind="ExternalInput").ap(),
    }
    d_out = {
        "out": nc.dram_tensor("out", [C, NPIX], F16,
                              kind="ExternalOutput").ap(),
    }
    with tile.TileContext(nc) as tc, ExitStack() as ctx:
        _emit(nc, tc, ctx, d_in, d_out)
    nc.compile()
    return nc


def _round_f32r(x):
    """Round fp32 to the FP32R-representable set (hi-bf16 + lo-bf16)."""
    import ml_dtypes

    x = np.asarray(x, np.float32)
    hi = x.astype(ml_dtypes.bfloat16).astype(np.float32)
    lo = (x - hi).astype(ml_dtypes.bfloat16).astype(np.float32)
    return hi + lo


def make_in_maps(a, b, w1, b1, w2, b2, w3, b3, w4, b4):
    N = NPIX
    f = np.float32
    A = (w2.astype(np.float64) @ w1.astype(np.float64)).astype(f)
    B_ = (w3.astype(np.float64) @ w1.astype(np.float64)).astype(f)
    Cw = (w4.astype(np.float64) @ w1.astype(np.float64)).astype(f)
    be2 = (w2 @ b1 + b2).astype(f)
    be3 = (w3 @ b1 + b3).astype(f)
    be4 = (w4 @ b1 + b4).astype(f)
    wcat = np.concatenate([A.T, B_.T, Cw], axis=1).astype(f, copy=False)
    brows = np.stack([be2, be3, N * be3]).astype(f, copy=False)
    bcols = np.ascontiguousarray(be4[:, None].astype(f))
    ident = np.eye(P, dtype=f)
    in_maps = []
    for i in range(a.shape[0]):
        x = np.concatenate([a[i].reshape(P, N), b[i].reshape(P, N)], axis=0)
        xh = x.astype(np.float16)
        xht = np.ascontiguousarray(
            xh.T.reshape(NPIECE, CH_PP, P, C).transpose(0, 2, 1, 3))
        ones = np.ones((NPIECE, P, CH_PP, 1), np.float16)
        xht = np.ascontiguousarray(np.concatenate([xht, ones], axis=3))
        in_maps.append({
            "xht": xht,
            "xnat": np.ascontiguousarray(xh.reshape(2, P, N)),
            "wcat": wcat,
            "brows": brows,
            "bcols": bcols,
            "ident": ident,
        })
    return in_maps


_CACHE = {}


def kernel(a, b, w1, b1, w2, b2, w3, b3, w4, b4, _trace=False):
    a = np.asarray(a, dtype=np.float32)
    b = np.asarray(b, dtype=np.float32)
    args = [np.asarray(t, dtype=np.float32)
            for t in (w1, b1, w2, b2, w3, b3, w4, b4)]
    if "nc" not in _CACHE:
        _CACHE["nc"] = build_program()
    nc = _CACHE["nc"]
    in_maps = make_in_maps(a, b, *args)
    res = run_bass_kernel_spmd(nc, in_maps, core_ids=list(range(8)),
                               trace=_trace)
    B, Ch, H, W = a.shape
    out = np.stack([np.asarray(r["out"], dtype=np.float32).reshape(C, H, W)
                    for r in res.results])
    if _trace:
        _CACHE["last_results"] = res
    return out
